# revision 2
# baseline (speedup 1.0000x reference)
"""Distributed sparse attention kernel for Trainium2 (8 NeuronCores) — v2.

Head-parallel: core c owns heads [2c, 2c+1] (128 of 1024 projection dims).

Per-core structure:
  P1 projections (per batch, per 512-token chunk):
     Q_scr bf16 [128, T]  <- bf16 matmuls of bf16 xq (screen only)
     KT    f32  [128, T]  <- 3-term bf16 hi/lo matmuls (err ~2.5e-4)
     K_scr bf16 [128, T]  <- bf16 copy of KT
     vsb   bf16 token-major V (bf16 matmuls + PE transposes)
     per-batch: exact ksum (DVE), vmean row, base row = vmean @ WoT summed
     over both heads, broadcast-filled into the fp16 partial (default rows).
  P2 importance screen per pair (h, b): scores = Q_scr.T @ K_scr (bf16
     matmuls, fp32 psum). h0 pairs: stat = ln(sum exp s) - mean (LSE proxy
     on the Act engine via fused accumulate); h1 pairs: max - mean (DVE).
     Screen err << gap(rank38 -> rank56), so top-56 covers the true top-38.
  P3 coarse top-56 via 7 rounds of DVE max8 on [8, S].
  P4 per pair: gather 56 raw xq rows -> exact fp32 Q re-projection -> exact
     fp32 rescore vs KT -> exact top-38-of-56 -> attention (f32r scores,
     Act exp, bf16 PV matmuls) -> correction rows delta = osel - vmean
     projected through WoT (f32r) -> scattered into the fp16 partial
     (head1 gather-add-scatter to handle token overlap with head0).
  P5 ReduceScatter (fp16, 2MB output) + bias add + fp32 out.
"""

import math
import sys

import numpy as np

sys.path.insert(0, "/opt/trn_rl_repo")

import concourse.bass as bass
import concourse.mybir as mybir
import concourse.tile as tile
from concourse import bacc
from concourse.masks import make_identity
from concourse.tile import add_dep_helper

F32 = mybir.dt.float32
F32R = mybir.dt.float32r
F16 = mybir.dt.float16
BF16 = mybir.dt.bfloat16
U32 = mybir.dt.uint32

B = 4
D = 1024
H = 16
HD = 64
H_LOC = 2
N_CORES = 8
U = 38           # exact top-k
UC = 56          # coarse candidates (7 rounds of max8)
US = 48          # exact-stage rounds (6 x 8); first 38 used
AF = mybir.ActivationFunctionType


def build_nc(S=2048, n_cores=8):
    nc = bacc.Bacc("TRN2", target_bir_lowering=False, debug=False,
                   num_devices=n_cores)
    T = B * S
    NP = min(512, S)          # projection chunk (tokens)
    NCHK = S // NP            # chunks per batch
    NQC = S // 128            # query chunks per pair
    SH = min(1024, S)         # scan half width (keys)
    NSH = S // SH             # halves per qc
    SST = min(512, SH)        # scan matmul step
    BW = min(512, SH)         # base-projection step
    ROWS_OUT = T // n_cores
    NRB = S // 128            # 128-row groups per batch
    scale = 1.0 / math.sqrt(HD)

    # ---- external inputs ----
    xq_scr_d = nc.dram_tensor("xq_scr", [D, T], BF16, kind="ExternalInput")
    xkh_d = nc.dram_tensor("xkh", [D, T], BF16, kind="ExternalInput")
    xkl_d = nc.dram_tensor("xkl", [D, T], BF16, kind="ExternalInput")
    xv_d = nc.dram_tensor("xv", [D, T], F16, kind="ExternalInput")
    xq_rm = nc.dram_tensor("xq_rm", [T, D], F32, kind="ExternalInput")
    wq_scr_d = nc.dram_tensor("wq_scr", [D, 128], BF16, kind="ExternalInput")
    wkh_d = nc.dram_tensor("wkh", [D, 128], BF16, kind="ExternalInput")
    wkl_d = nc.dram_tensor("wkl", [D, 128], BF16, kind="ExternalInput")
    wv_d = nc.dram_tensor("wv", [D, 128], F16, kind="ExternalInput")
    wq_f32_d = nc.dram_tensor("wq_f32", [D, 128], F32, kind="ExternalInput")
    bq_d = nc.dram_tensor("bq", [128, 1], F32, kind="ExternalInput")
    bk_d = nc.dram_tensor("bk", [128, 1], F32, kind="ExternalInput")
    bv_d = nc.dram_tensor("bv", [128, 1], F32, kind="ExternalInput")
    woT_d = nc.dram_tensor("woT", [128, D], F32R, kind="ExternalInput")
    bo_d = nc.dram_tensor("boN", [1, D], F32, kind="ExternalInput")
    boff_d = nc.dram_tensor("boff", [4, 2], U32, kind="ExternalInput")
    seg_off_d = nc.dram_tensor("seg_off", [16, 1], U32, kind="ExternalInput")

    # ---- DRAM scratch / output ----
    partial = nc.dram_tensor("partial", [T, D], F16)
    rs_out = nc.dram_tensor("rs_out", [ROWS_OUT, D], F16)
    idxtok_dram = nc.dram_tensor("idxtok", [8 * UC, 1], U32)
    selstage_dram = nc.dram_tensor("selstage", [8 * US], U32)
    idxcat_dram = nc.dram_tensor("idxcat", [8 * 4 * UC, 1], U32)
    imps_dram = nc.dram_tensor("imps", [8 * S], F32)
    dsb_dram = nc.dram_tensor("dsb_dram", [8 * UC, D], F16)
    out_ext = nc.dram_tensor("out", [ROWS_OUT, D], F32, kind="ExternalOutput")

    with tile.TileContext(nc) as tc:
        with (
            tc.tile_pool(name="res", bufs=1) as res,
            tc.tile_pool(name="consts", bufs=1) as consts,
        ):
            ident = consts.tile([128, 128], F32)
            make_identity(nc, ident[:])
            ident_bf = consts.tile([128, 128], F16)
            nc.vector.tensor_copy(ident_bf[:], ident[:])
            ones_row_f = consts.tile([1, 128], F32)
            nc.vector.memset(ones_row_f[:], 1.0)
            ones_col_f = consts.tile([128, 1], F32)
            nc.vector.memset(ones_col_f[:], 1.0)
            ones_col_bf = consts.tile([128, 1], F16)
            nc.vector.tensor_copy(ones_col_bf[:], ones_col_f[:])
            ones_row_r = consts.tile([1, 128], F32R)
            nc.vector.tensor_copy(ones_row_r[:], ones_row_f[:])

            # resident weights / biases
            wq_sb = res.tile([128, 8, 128], BF16)
            wkh_sb = res.tile([128, 8, 128], BF16)
            wkl_sb = res.tile([128, 8, 128], BF16)
            wv_sb = res.tile([128, 8, 128], F16)
            for dst, src in ((wq_sb, wq_scr_d), (wkh_sb, wkh_d),
                             (wkl_sb, wkl_d), (wv_sb, wv_d)):
                nc.sync.dma_start(out=dst[:],
                                  in_=src[:].rearrange("(k p) m -> p k m", p=128))
            wo_sb = res.tile([128, D], F32R)
            nc.sync.dma_start(out=wo_sb[:], in_=woT_d[:])
            wo_h = []
            for hh in range(H_LOC):
                woh = res.tile([64, D], F32R, tag=f"wo_h{hh}")
                nc.sync.dma_start(
                    out=woh[:],
                    in_=woT_d[hh * 64:(hh + 1) * 64, :])
                wo_h.append(woh)
            bq_sb = consts.tile([128, 1], F32)
            bk_sb = consts.tile([128, 1], F32)
            bv_sb = consts.tile([128, 1], F32)
            nc.sync.dma_start(out=bq_sb[:], in_=bq_d[:])
            nc.sync.dma_start(out=bk_sb[:], in_=bk_d[:])
            nc.sync.dma_start(out=bv_sb[:], in_=bv_d[:])
            bo_sb = consts.tile([1, D], F32)
            nc.sync.dma_start(out=bo_sb[:], in_=bo_d[:])
            boff_sb = consts.tile([4, 2], U32)
            nc.sync.dma_start(out=boff_sb[:], in_=boff_d[:])
            seg_off_sb = consts.tile([16, 1], U32)
            nc.sync.dma_start(out=seg_off_sb[:], in_=seg_off_d[:])

            # residents
            Qs = res.tile([128, T], BF16)        # screen Q (dims x tokens)
            KT = res.tile([128, T], F32)         # exact-ish K
            Ks = res.tile([128, T], BF16)        # screen K
            vsb = res.tile([128, NRB * B, 128], F16)  # token-major V
            ksum = res.tile([128, B], F32)       # exact per-batch K col sums
            ksum_bf = res.tile([128, B], BF16)
            imp_all = res.tile([128, 8 * NQC], F32)
            vmr = res.tile([1, B, 128], F32)     # per-batch V means (2 heads)
            base_bc = res.tile([128, B, D], F16)  # base row bcast (per batch)

            # ---------- P1 + P2: projections and screen, per batch ----------
            fills_by_batch = {}
            with (
                tc.tile_pool(name="xin", bufs=2) as xin,
                tc.tile_pool(name="pcopy", bufs=3) as pcopy,
                tc.tile_pool(name="ps_proj", bufs=2, space="PSUM") as psp,
                tc.tile_pool(name="ps_tr", bufs=1, space="PSUM") as pstr0,
                tc.tile_pool(name="ps_scan", bufs=2, space="PSUM") as pss,
                tc.tile_pool(name="ps_mean", bufs=1, space="PSUM") as psm,
                tc.tile_pool(name="scan_sb", bufs=3) as ssb,
                tc.tile_pool(name="scan_jk", bufs=2) as ssbj,
                tc.tile_pool(name="scan_b1", bufs=1) as ssb1,
            ):
                def emit_proj_chunk(b, c):
                    sl = slice(b * S + c * NP, b * S + (c + 1) * NP)
                    # --- screen Q ---
                    xt = xin.tile([128, 8, NP], BF16, tag="xq")
                    nc.sync.dma_start(
                        out=xt[:],
                        in_=xq_scr_d[:, sl].rearrange("(k p) t -> p k t", p=128))
                    ps = psp.tile([128, NP], F32, tag="pp")
                    for kc in range(8):
                        nc.tensor.matmul(ps[:], lhsT=wq_sb[:, kc, :],
                                         rhs=xt[:, kc, :],
                                         start=(kc == 0), stop=(kc == 7))
                    nc.scalar.activation(Qs[:, sl], ps[:], AF.Identity,
                                         bias=bq_sb[:])
                    # --- exact-ish K (3-term bf16) ---
                    xh = xin.tile([128, 8, NP], BF16, tag="xkh")
                    xl = xin.tile([128, 8, NP], BF16, tag="xkl")
                    nc.sync.dma_start(
                        out=xh[:],
                        in_=xkh_d[:, sl].rearrange("(k p) t -> p k t", p=128))
                    nc.sync.dma_start(
                        out=xl[:],
                        in_=xkl_d[:, sl].rearrange("(k p) t -> p k t", p=128))
                    psk = psp.tile([128, NP], F32, tag="pp")
                    for kc in range(8):
                        nc.tensor.matmul(psk[:], lhsT=wkh_sb[:, kc, :],
                                         rhs=xh[:, kc, :],
                                         start=(kc == 0), stop=False)
                        nc.tensor.matmul(psk[:], lhsT=wkh_sb[:, kc, :],
                                         rhs=xl[:, kc, :], start=False, stop=False)
                        nc.tensor.matmul(psk[:], lhsT=wkl_sb[:, kc, :],
                                         rhs=xh[:, kc, :],
                                         start=False, stop=(kc == 7))
                    nc.scalar.activation(KT[:, sl], psk[:], AF.Identity,
                                         bias=bk_sb[:])
                    nc.vector.tensor_copy(Ks[:, sl], KT[:, sl])
                    # --- V (token-major, bf16) ---
                    xv = xin.tile([128, 8, NP], F16, tag="xv")
                    nc.sync.dma_start(
                        out=xv[:],
                        in_=xv_d[:, sl].rearrange("(k p) t -> p k t", p=128))
                    psv = psp.tile([128, NP], F32, tag="pp")
                    for kc in range(8):
                        nc.tensor.matmul(psv[:], lhsT=wv_sb[:, kc, :],
                                         rhs=xv[:, kc, :],
                                         start=(kc == 0), stop=(kc == 7))
                    vtmp = pcopy.tile([128, NP], F16, tag="vt")
                    nc.scalar.activation(vtmp[:], psv[:], AF.Identity,
                                         bias=bv_sb[:])
                    for j in range(NP // 128):
                        pt = pstr0.tile([128, 128], F16, tag="ptv")
                        nc.tensor.transpose(pt[:],
                                            in_=vtmp[:, j * 128:(j + 1) * 128],
                                            identity=ident_bf[:])
                        ci = (b * S + c * NP) // 128 + j
                        nc.scalar.copy(vsb[:, ci, :], pt[:])

                def emit_batch_stats(b):
                    """ksum, vmean, base row, default-fill of partial rows."""
                    bs = slice(b * S, (b + 1) * S)
                    nc.vector.reduce_sum(ksum[:, b:b + 1], KT[:, bs],
                                         axis=mybir.AxisListType.X)
                    nc.vector.tensor_copy(ksum_bf[:, b:b + 1], ksum[:, b:b + 1])
                    st = pss.tile([128, SH], F32, tag="sc")
                    # vmean over tokens (both heads at once)
                    pvm = st[0:1, 0:128]
                    for ci in range(NRB):
                        nc.tensor.matmul(pvm, lhsT=ones_col_bf[:],
                                         rhs=vsb[:, b * NRB + ci, :],
                                         start=(ci == 0), stop=(ci == NRB - 1))
                    nc.vector.tensor_scalar_mul(vmr[:, b, :], pvm, 1.0 / S)
                    # vmrT column for the base projection
                    pvt = st[0:128, 130:131]
                    nc.tensor.transpose(pvt, in_=vmr[:, b, :],
                                        identity=ident[0:1, 0:1])
                    vmrT = ssb1.tile([128, 1], F32R, tag="vmrT")
                    nc.scalar.copy(vmrT[:], pvt)
                    # base row = vmrT.T @ WoT (both heads summed), bcast, fill
                    base_sb = ssb1.tile([1, D], F32R, tag="base")
                    for nh in range(D // BW):
                        nsl = slice(nh * BW, (nh + 1) * BW)
                        pb = st[0:1, 0:BW]
                        nc.tensor.matmul(pb, lhsT=vmrT[:], rhs=wo_sb[:, nsl],
                                         start=True, stop=True)
                        nc.scalar.copy(base_sb[:, nsl], pb)
                    for nh in range(D // BW):
                        nsl = slice(nh * BW, (nh + 1) * BW)
                        pbc = st[0:128, 0:BW]
                        nc.tensor.matmul(pbc, lhsT=ones_row_r[:],
                                         rhs=base_sb[:, nsl],
                                         start=True, stop=True)
                        nc.scalar.copy(base_bc[:, b, nsl], pbc)
                    fill = nc.sync.dma_start(
                        out=partial[bs, :].rearrange("(r p) d -> p r d", p=128),
                        in_=base_bc[:, b:b + 1, :].broadcast_to([128, NRB, D]))
                    return [fill]

                def emit_scan_qc(b, h, qc):
                    pair = b * H_LOC + h
                    hsl = slice(h * 64, (h + 1) * 64)
                    qsl = slice(b * S + qc * 128, b * S + (qc + 1) * 128)
                    use_lse = (h == 0)
                    pm = psm.tile([128, 1], F32, tag="pmean")
                    nc.tensor.matmul(pm[:], lhsT=Qs[hsl, qsl],
                                     rhs=ksum_bf[hsl, b:b + 1],
                                     start=True, stop=True)
                    stat = ssb.tile([128, NSH], F32, tag="stat")
                    for half in range(NSH):
                        sc = pss.tile([128, SH], F32, tag="sc")
                        for j in range(SH // SST):
                            jsl = slice(b * S + half * SH + j * SST,
                                        b * S + half * SH + (j + 1) * SST)
                            nc.tensor.matmul(sc[:, j * SST:(j + 1) * SST],
                                             lhsT=Qs[hsl, qsl], rhs=Ks[hsl, jsl],
                                             start=True, stop=True)
                        if use_lse:
                            junk = ssbj.tile([128, SH], BF16, tag="junk")
                            nc.scalar.activation(junk[:], sc[:], AF.Exp,
                                                 accum_out=stat[:, half:half + 1])
                        else:
                            nc.vector.reduce_max(stat[:, half:half + 1], sc[:],
                                                 axis=mybir.AxisListType.X)
                    col = slice(pair * NQC + qc, pair * NQC + qc + 1)
                    red = ssb.tile([128, 1], F32, tag="red")
                    if NSH > 1:
                        nc.vector.tensor_reduce(
                            red[:], stat[:], axis=mybir.AxisListType.X,
                            op=(mybir.AluOpType.add if use_lse
                                else mybir.AluOpType.max))
                    else:
                        nc.vector.tensor_copy(red[:], stat[:])
                    mn = ssb.tile([128, 1], F32, tag="mn")
                    if use_lse:
                        # rank by sumexp * exp(-mean): monotone in LSE - mean
                        # (HW Ln is inaccurate for inputs beyond ~1e17)
                        nc.scalar.activation(mn[:], pm[:], AF.Exp, scale=-1.0 / S)
                        nc.vector.tensor_mul(imp_all[:, col], red[:], mn[:])
                    else:
                        nc.vector.tensor_scalar_mul(mn[:], pm[:], 1.0 / S)
                        nc.vector.tensor_sub(imp_all[:, col], red[:], mn[:])

                off_t = [None] * 8   # per-pair [UC,1] u32 global offsets
                idx_wr_by_g = {}

                NSEG = 4
                SEG = S // NSEG

                def emit_topk(g, pst_ap, sb_pool):
                    """Coarse top-56 for pair rows [4g, 4g+4) (batches
                    2g, 2g+1), hierarchically: per-segment top-56 on
                    [16, SEG], then top-56 of the merged [4, 4*56]."""
                    NQ4 = 4 * NQC
                    cols = slice(g * 4 * NQC, (g + 1) * 4 * NQC)
                    nc.tensor.transpose(pst_ap[0:NQ4, 0:128],
                                        in_=imp_all[:, cols],
                                        identity=ident[:])
                    impT = sb_pool.tile([4 * NQC, 128], F32, tag="impT")
                    nc.scalar.copy(impT[:], pst_ap[0:NQ4, 0:128])
                    impP = sb_pool.tile([16, SEG], F32, tag="impP")
                    isl = slice(g * 4 * S, (g + 1) * 4 * S)
                    iw1 = nc.sync.dma_start(out=imps_dram[isl], in_=impT[:])
                    rgr = nc.sync.dma_start(out=impP[:], in_=imps_dram[isl])
                    add_dep_helper(rgr.ins, iw1.ins, sync=True,
                                   reason="impP read after flat write")
                    mxv = sb_pool.tile([16, UC], F32, tag="mxv")
                    idx = sb_pool.tile([16, UC], U32, tag="idx")
                    for r in range(UC // 8):
                        rsl = slice(r * 8, (r + 1) * 8)
                        nc.vector.max(out=mxv[:, rsl], in_=impP[:])
                        nc.vector.max_index(out=idx[:, rsl], in_max=mxv[:, rsl],
                                            in_values=impP[:])
                        if r < UC // 8 - 1:
                            nc.vector.match_replace(out=impP[:],
                                                    in_to_replace=mxv[:, rsl],
                                                    in_values=impP[:],
                                                    imm_value=-1e30)
                    nc.vector.tensor_tensor(
                        idx[:], idx[:], seg_off_sb[:].to_broadcast([16, UC]),
                        op=mybir.AluOpType.add)
                    # merge the 4 segments of each pair
                    mxc = sb_pool.tile([4, NSEG * UC], F32, tag="mxc")
                    idxc = sb_pool.tile([4, NSEG * UC], U32, tag="idxc")
                    cwrs = []
                    for pr in range(4):
                        nc.gpsimd.dma_start(out=mxc[pr:pr + 1, :],
                                            in_=mxv[pr * 4:(pr + 1) * 4, :])
                        cwrs.append(nc.gpsimd.dma_start(
                            out=idxc[pr:pr + 1, :],
                            in_=idx[pr * 4:(pr + 1) * 4, :]))
                    icw = nc.gpsimd.dma_start(
                        out=idxcat_dram[g * 4 * NSEG * UC:
                                        (g + 1) * 4 * NSEG * UC, :],
                        in_=idxc[:])
                    for cw in cwrs:
                        add_dep_helper(icw.ins, cw.ins, sync=True,
                                       reason="idxcat after merge")
                    mx2 = sb_pool.tile([4, UC], F32, tag="mx2c")
                    pos2 = sb_pool.tile([4, UC], U32, tag="pos2c")
                    for r in range(UC // 8):
                        rsl = slice(r * 8, (r + 1) * 8)
                        nc.vector.max(out=mx2[:, rsl], in_=mxc[:])
                        nc.vector.max_index(out=pos2[:, rsl],
                                            in_max=mx2[:, rsl],
                                            in_values=mxc[:])
                        if r < UC // 8 - 1:
                            nc.vector.match_replace(out=mxc[:],
                                                    in_to_replace=mx2[:, rsl],
                                                    in_values=mxc[:],
                                                    imm_value=-1e30)
                    for pr in range(4):
                        pair = g * 4 + pr
                        b = pair // H_LOC
                        pc = res.tile([UC, 1], U32, tag=f"pc{pair}")
                        nc.gpsimd.dma_start(out=pc[:],
                                            in_=pos2[pr:pr + 1, :])
                        nc.vector.tensor_scalar_add(pc[:], pc[:],
                                                    pair * NSEG * UC)
                        ot = res.tile([UC, 1], U32, tag=f"ot{pair}")
                        otg = nc.gpsimd.indirect_dma_start(
                            out=ot[:], out_offset=None, in_=idxcat_dram[:],
                            in_offset=bass.IndirectOffsetOnAxis(
                                ap=pc[:, 0:1], axis=0))
                        add_dep_helper(otg.ins, icw.ins, sync=True,
                                       reason="candidate gather after idxcat")
                        nc.vector.tensor_scalar_add(ot[:], ot[:], b * S)
                        off_t[pair] = ot
                        iw = nc.gpsimd.dma_start(
                            out=idxtok_dram[pair * UC:(pair + 1) * UC, :],
                            in_=ot[:])
                        idx_wr_by_g[(g, pr)] = iw

                # bo broadcast rows (used in the per-batch final stage),
                # via a scan-psum tile before the heavy phase
                bo_bc = res.tile([128, D], F32, tag="bo_bc")
                st0 = pss.tile([128, SH], F32, tag="sc")
                for nh in range(D // BW):
                    nsl = slice(nh * BW, (nh + 1) * BW)
                    nc.tensor.matmul(st0[0:128, 0:BW], lhsT=ones_row_f[:],
                                     rhs=bo_sb[:, nsl], start=True, stop=True)
                    nc.scalar.copy(bo_bc[:, nsl], st0[0:128, 0:BW])

                for b in range(B):
                    for c in range(NCHK):
                        emit_proj_chunk(b, c)
                    fills_by_batch[b] = emit_batch_stats(b)
                    for qc in range(NQC):
                        for h in range(H_LOC):
                            emit_scan_qc(b, h, qc)
                    if b == 1:
                        sttk = pss.tile([128, SH], F32, tag="sc")
                        emit_topk(0, sttk[:], ssb1)

            # ---------- P4: rescore + attention + corrections ----------
            SH2 = min(1024, S)
            NSH2 = S // SH2
            scat_last = {}
            cc_by_batch = {}
            with (
                tc.tile_pool(name="ps_rsc", bufs=1, space="PSUM") as psrc,
                tc.tile_pool(name="ps_dp", bufs=1, space="PSUM") as psdp,
                tc.tile_pool(name="ps_t", bufs=2, space="PSUM") as pst2p,
                tc.tile_pool(name="ps_sm", bufs=2, space="PSUM") as ps2,
                tc.tile_pool(name="ps_tk2", bufs=1, space="PSUM") as pstk2,
                tc.tile_pool(name="att_sb", bufs=2) as asb,
                tc.tile_pool(name="idx_sb", bufs=8) as isb,
                tc.tile_pool(name="tk2_sb", bufs=1) as tk2sb,
            ):
                wqf_sb = tk2sb.tile([128, 8, 128], F32, tag="wqf")
                nc.sync.dma_start(
                    out=wqf_sb[:],
                    in_=wq_f32_d[:].rearrange("(k p) m -> p k m", p=128))
                zrow_f = tk2sb.tile([128, 1, D], F32, tag="zrow_f")
                nc.vector.memset(zrow_f[:], 0.0)
                zrow = tk2sb.tile([128, 1, D], F16, tag="zrow")
                nc.vector.tensor_copy(zrow[:], zrow_f[:])
                nc.sync.dma_start(
                    out=dsb_dram[:].rearrange("(r p) d -> p r d", p=64),
                    in_=zrow[0:64, :, :].broadcast_to([64, 8 * UC // 64, D]))
                def emit_p4(b, h):
                    if True:
                        pair = b * H_LOC + h
                        hsl = slice(h * 64, (h + 1) * 64)
                        off = off_t[pair]
                        # gather 56 raw query rows, transpose to [128, 8, UC]
                        xcand = asb.tile([UC, D], F32, tag="xcand")
                        nc.gpsimd.indirect_dma_start(
                            out=xcand[:], out_offset=None, in_=xq_rm[:],
                            in_offset=bass.IndirectOffsetOnAxis(
                                ap=off[:, 0:1], axis=0))
                        xcT = asb.tile([128, 8, UC], F32, tag="xcT")
                        for kc in range(8):
                            ptx = pst2p.tile([128, UC], F32, tag="t128c")
                            nc.tensor.transpose(
                                ptx[:], in_=xcand[:, kc * 128:(kc + 1) * 128],
                                identity=ident[0:UC, 0:UC])
                            nc.scalar.copy(xcT[:, kc, :], ptx[:])
                        # exact Q for candidates
                        smq = ps2.tile([128, 64], F32, tag="sm")
                        pq = smq[0:64, 0:UC]
                        for kc in range(8):
                            nc.tensor.matmul(pq, lhsT=wqf_sb[:, kc, hsl],
                                             rhs=xcT[:, kc, :],
                                             start=(kc == 0), stop=(kc == 7))
                        qselTf = asb.tile([64, UC], F32, tag="qselT")
                        qselT = qselTf[:]
                        nc.scalar.activation(qselT, pq, AF.Identity,
                                             bias=bq_sb[hsl])
                        # head-1 operands must move to partition base 0 (matmul
                        # lhsT/rhs bases must match; psum outs must be base 0)
                        if h == 0:
                            ktb0 = None
                            ksb0_ap = ksum[0:64, b:b + 1]
                        else:
                            ktb0 = asb.tile([64, S], F32, tag="ktb0")
                            nc.sync.dma_start(out=ktb0[:],
                                              in_=KT[hsl, b * S:(b + 1) * S])
                            ksb0 = asb.tile([64, 1], F32, tag="ksb0")
                            nc.sync.dma_start(out=ksb0[:],
                                              in_=ksum[hsl, b:b + 1])
                            ksb0_ap = ksb0[:]
                        # exact rescore vs full K
                        xm = asb.tile([UC, NSH2], F32, tag="xm")
                        for half in range(NSH2):
                            psc = psrc.tile([UC, SH2], F32, tag="psc")
                            for j in range(SH2 // SST):
                                lo = half * SH2 + j * SST
                                kap = (KT[0:64, b * S + lo:b * S + lo + SST]
                                       if h == 0 else ktb0[:, lo:lo + SST])
                                nc.tensor.matmul(psc[:, j * SST:(j + 1) * SST],
                                                 lhsT=qselT, rhs=kap,
                                                 start=True, stop=True)
                            nc.vector.reduce_max(xm[:, half:half + 1], psc[:],
                                                 axis=mybir.AxisListType.X)
                        smm = ps2.tile([128, 64], F32, tag="sm")
                        pm2 = smm[0:UC, 0:1]
                        nc.tensor.matmul(pm2, lhsT=qselT, rhs=ksb0_ap,
                                         start=True, stop=True)
                        impc = asb.tile([UC, 1], F32, tag="impc")
                        nc.vector.tensor_scalar_mul(impc[:], pm2, 1.0 / S)
                        xmx = asb.tile([UC, 1], F32, tag="xmx")
                        if NSH2 > 1:
                            nc.vector.tensor_reduce(xmx[:], xm[:],
                                                    axis=mybir.AxisListType.X,
                                                    op=mybir.AluOpType.max)
                        else:
                            nc.vector.tensor_copy(xmx[:], xm[:])
                        nc.vector.tensor_sub(impc[:], xmx[:], impc[:])
                        impcT = isb.tile([1, UC], F32, tag="impcT")
                        nc.sync.dma_start(out=impcT[:], in_=impc[:])
                        # exact top-38 (5 rounds of max8 over 56)
                        mx2 = isb.tile([1, US], F32, tag="mx2")
                        pos = isb.tile([1, US], U32, tag="pos")
                        wk2 = isb.tile([1, UC], F32, tag="wk2")
                        nc.vector.tensor_copy(wk2[:], impcT[:])
                        for r in range(US // 8):
                            rsl = slice(r * 8, (r + 1) * 8)
                            nc.vector.max(out=mx2[:, rsl], in_=wk2[:])
                            nc.vector.max_index(out=pos[:, rsl],
                                                in_max=mx2[:, rsl],
                                                in_values=wk2[:])
                            if r < US // 8 - 1:
                                nc.vector.match_replace(
                                    out=wk2[:], in_to_replace=mx2[:, rsl],
                                    in_values=wk2[:], imm_value=-1e30)
                        posg48 = isb.tile([US, 1], U32, tag="posg48")
                        nc.sync.dma_start(out=posg48[:], in_=pos[:, 0:US])
                        nc.vector.tensor_scalar_add(posg48[:], posg48[:],
                                                    pair * UC)
                        posg = posg48[0:U, 0:1]
                        seltok48 = isb.tile([US, 1], U32, tag="seltok48")
                        stg = nc.gpsimd.indirect_dma_start(
                            out=seltok48[:], out_offset=None,
                            in_=idxtok_dram[:],
                            in_offset=bass.IndirectOffsetOnAxis(
                                ap=posg48[:, 0:1], axis=0))
                        add_dep_helper(stg.ins,
                                       idx_wr_by_g[(pair // 4, pair % 4)].ins,
                                       sync=True,
                                       reason="seltok gather after idx write")
                        # dma_scatter_add wants int16 indices wrapped as
                        # [i % 16, i // 16], replicated on all 8 Q7 cores
                        # (128 partitions). Stage 8 copies in DRAM, read back
                        # in the wrapped layout.
                        NS16 = US // 16
                        ssl = slice(pair * US, (pair + 1) * US)
                        # stage A: write tokens permuted so linear order is
                        # the wrapped [i % 16, i // 16] layout
                        stw = nc.sync.dma_start(
                            out=selstage_dram[ssl].rearrange(
                                "(pl s) -> s pl", s=NS16),
                            in_=seltok48[:, 0])
                        # stage B: replicate to all 8 Q7 cores + cast to int16
                        idx128 = res.tile([128, NS16], mybir.dt.int16,
                                          tag=f"idx128_{pair}")
                        idxr = nc.gpsimd.dma_start(
                            out=idx128[:],
                            in_=selstage_dram[ssl].unsqueeze(0).broadcast_to(
                                [8, US]))
                        add_dep_helper(idxr.ins, stw.ins, sync=True,
                                       reason="idx read after stage write")

                        # attention over all 56 candidates
                        expT = asb.tile([128, NRB, UC], F16, tag="expT")

                        for kc in range(NRB):
                            kap = (KT[0:64, b * S + kc * 128:b * S + (kc + 1) * 128]
                                   if h == 0 else ktb0[:, kc * 128:(kc + 1) * 128])
                            pst2 = pst2p.tile([128, UC], F32, tag="t128c")
                            nc.tensor.matmul(pst2[:], lhsT=kap, rhs=qselT,
                                             start=True, stop=True)
                            nc.scalar.activation(expT[:, kc, :], pst2[:], AF.Exp,
                                                 scale=scale)
                        sma = ps2.tile([128, 64], F32, tag="sm")
                        pse = sma[0:UC, 0:1]
                        smb = ps2.tile([128, 64], F32, tag="sm")
                        pot = smb[0:64, 0:UC]
                        for kc in range(NRB):
                            nc.tensor.matmul(pse, lhsT=expT[:, kc, :],
                                             rhs=ones_col_bf[:],
                                             start=(kc == 0), stop=(kc == NRB - 1))
                            nc.tensor.matmul(pot, lhsT=vsb[:, b * NRB + kc, hsl],
                                             rhs=expT[:, kc, :],
                                             start=(kc == 0), stop=(kc == NRB - 1))
                        se = asb.tile([UC, 1], F32, tag="se")
                        nc.vector.tensor_scalar_add(se[:], pse, 1e-8)
                        rec = asb.tile([UC, 1], F32, tag="rec")
                        nc.vector.reciprocal(rec[:], se[:])
                        oT = asb.tile([64, UC], F32, tag="oT")
                        nc.scalar.copy(oT[:], pot)
                        smc = ps2.tile([128, 64], F32, tag="sm")
                        po = smc[0:UC, 0:64]
                        nc.tensor.transpose(po, in_=oT[:],
                                            identity=ident[0:64, 0:64])
                        osel = asb.tile([UC, HD], F32, tag="osel")
                        nc.scalar.mul(osel[:], po, rec[:, 0:1])
                        smd = ps2.tile([128, 64], F32, tag="sm")
                        pvb = smd[0:UC, 0:64]
                        nc.tensor.matmul(pvb, lhsT=ones_row_f[:, 0:UC],
                                         rhs=vmr[:, b, hsl],
                                         start=True, stop=True)
                        delta = asb.tile([UC, HD], F32, tag="delta")
                        nc.vector.tensor_sub(delta[:], osel[:], pvb)
                        sme = ps2.tile([128, 64], F32, tag="sm")
                        pdT = sme[0:64, 0:UC]
                        nc.tensor.transpose(pdT, in_=delta[:],
                                            identity=ident[0:UC, 0:UC])
                        deltaTf = asb.tile([64, UC], F32R, tag="deltaT")
                        deltaT = deltaTf[:]
                        nc.scalar.copy(deltaT, pdT)
                        # dproj[cand, D] = delta @ WoT_h  (f32r), 2 halves
                        # through a single-bank psum tile
                        dsb = asb.tile([UC, D], F16, tag="dsb")
                        for nh in range(D // 512):
                            nsl = slice(nh * 512, (nh + 1) * 512)
                            pdp = psdp.tile([UC, 512], F32, tag="pdp")
                            nc.tensor.matmul(pdp[:], lhsT=deltaT,
                                             rhs=wo_h[h][:, nsl],
                                             start=True, stop=True)
                            nc.vector.tensor_copy(dsb[:, nsl], pdp[:])
                        # stage all candidate rows to DRAM, gather the 38
                        # selected (indirect DMA can't gather from SBUF)
                        dwr = nc.sync.dma_start(
                            out=dsb_dram[pair * UC:(pair + 1) * UC, :], in_=dsb[:])
                        dsel = asb.tile([128, 1, D], F16, tag="dsel")
                        nc.vector.tensor_copy(dsel[:], zrow[:])
                        gsel = nc.gpsimd.indirect_dma_start(
                            out=dsel[0:U, 0, :], out_offset=None,
                            in_=dsb_dram[:],
                            in_offset=bass.IndirectOffsetOnAxis(
                                ap=posg[:, 0:1], axis=0))
                        add_dep_helper(gsel.ins, dwr.ins, sync=True,
                                       reason="gather after dsb write")
                        # scatter-ADD delta rows onto the base-filled partial;
                        # add semantics make h0/h1 token overlap commute
                        scat = nc.gpsimd.dma_scatter_add(
                            partial[:], dsel[:], idx128[:], U, U, D)
                        for f in fills_by_batch[b]:
                            add_dep_helper(scat.ins, f.ins, sync=True,
                                           reason="scatter-add after base fill")
                        if b in scat_last:
                            add_dep_helper(scat.ins, scat_last[b].ins, sync=True,
                                           reason="serialize adds per batch")
                        scat_last[b] = scat
                def emit_cc(b):
                    # per-batch reduce-scatter: fires as soon as this batch's
                    # scatters land, overlapping later batches' compute.
                    # rs_out piece b holds rows [b*PR, (b+1)*PR) = this core's
                    # slice of batch b (host reassembles).
                    PR = S // n_cores
                    cc = nc.gpsimd.collective_compute(
                        "ReduceScatter",
                        mybir.AluOpType.add,
                        replica_groups=[list(range(n_cores))],
                        ins=[partial[b * S:(b + 1) * S, :]],
                        outs=[rs_out[b * PR:(b + 1) * PR, :]],
                    )
                    add_dep_helper(cc.ins, scat_last[b].ins, sync=True,
                                   reason="RS after batch scatters")
                    cc_by_batch[b] = cc

                for h in range(H_LOC):
                    emit_p4(0, h)
                emit_cc(0)
                pstk2t = pstk2.tile([64, 128], F32, tag="ptk2")
                emit_topk(1, pstk2t[:], tk2sb)
                for b in (1, 2, 3):
                    for h in range(H_LOC):
                        emit_p4(b, h)
                    emit_cc(b)

            # ---------------- P5: per-batch bias add + output ----------------
            with (
                tc.tile_pool(name="fin", bufs=2) as fin,
            ):
                PR = S // n_cores
                NR = PR // 128
                for b in range(B):
                    rsl = slice(b * PR, (b + 1) * PR)
                    ft = fin.tile([128, max(NR, 1), D], F16, tag="ft")
                    if NR >= 1:
                        ld = nc.sync.dma_start(
                            out=ft[:, 0:NR, :],
                            in_=rs_out[rsl, :].rearrange("(r p) d -> p r d",
                                                         p=128))
                    else:
                        ld = nc.sync.dma_start(
                            out=ft[0:PR, 0, :], in_=rs_out[rsl, :])
                    add_dep_helper(ld.ins, cc_by_batch[b].ins, sync=True,
                                   reason="load after batch RS")
                    fo = fin.tile([128, max(NR, 1), D], F32, tag="fo")
                    for r in range(max(NR, 1)):
                        nc.gpsimd.tensor_tensor(fo[:, r, :], ft[:, r, :],
                                                bo_bc[:],
                                                op=mybir.AluOpType.add)
                    if NR >= 1:
                        nc.sync.dma_start(
                            out=out_ext[rsl, :].rearrange("(r p) d -> p r d",
                                                          p=128),
                            in_=fo[:, 0:NR, :])
                    else:
                        nc.sync.dma_start(out=out_ext[rsl, :],
                                          in_=fo[0:PR, 0, :])


    nc.finalize()
    return nc


def _prep_host_inputs(queries, keys, values, Wq, bq, Wk, bk, Wv, bv, Wo, bo,
                      S, n_cores):
    import ml_dtypes
    BF = ml_dtypes.bfloat16
    T = B * S
    xq = np.ascontiguousarray(queries.reshape(T, D).astype(np.float32))
    xqT = np.ascontiguousarray(xq.T)
    xkT = np.ascontiguousarray(keys.reshape(T, D).T.astype(np.float32))
    xvT = np.ascontiguousarray(values.reshape(T, D).T.astype(np.float32))
    xq_scr = xqT.astype(BF)
    xkh = xkT.astype(BF)
    xkl = (xkT - xkh.astype(np.float32)).astype(BF)
    xv_bf = xvT.astype(np.float16)
    boff = np.array([[((g * 4 + pr) // H_LOC) * S for g in range(2)]
                     for pr in range(4)], np.uint32)
    seg_off = np.array([(p % 4) * (S // 4) for p in range(16)],
                       np.uint32).reshape(16, 1)
    in_maps = []
    for c in range(n_cores):
        rsl = slice(c * 128, (c + 1) * 128)
        wq_s = np.ascontiguousarray(Wq[rsl, :].T.astype(np.float32))
        wk_s = np.ascontiguousarray(Wk[rsl, :].T.astype(np.float32))
        wv_s = np.ascontiguousarray(Wv[rsl, :].T.astype(np.float32))
        wkh = wk_s.astype(BF)
        wkl = (wk_s - wkh.astype(np.float32)).astype(BF)
        in_maps.append({
            "xq_scr": xq_scr, "xkh": xkh, "xkl": xkl, "xv": xv_bf,
            "xq_rm": xq,
            "wq_scr": wq_s.astype(BF), "wkh": wkh, "wkl": wkl,
            "wv": wv_s.astype(np.float16), "wq_f32": wq_s,
            "bq": bq[rsl].reshape(128, 1).astype(np.float32),
            "bk": bk[rsl].reshape(128, 1).astype(np.float32),
            "bv": bv[rsl].reshape(128, 1).astype(np.float32),
            "woT": np.ascontiguousarray(Wo.T[rsl, :].astype(np.float32)),
            "boN": bo.reshape(1, D).astype(np.float32),
            "boff": boff, "seg_off": seg_off,
        })
    return in_maps


_LAST_RESULT = None


def kernel(queries, keys, values, Wq, bq, Wk, bk, Wv, bv, Wo, bo):
    global _LAST_RESULT
    from concourse.bass_utils import run_bass_kernel_spmd

    queries, keys, values = (np.asarray(t, np.float32) for t in
                             (queries, keys, values))
    Wq, bq, Wk, bk, Wv, bv, Wo, bo = (np.asarray(t, np.float32) for t in
                                      (Wq, bq, Wk, bk, Wv, bv, Wo, bo))
    S = queries.shape[1]
    n_cores = N_CORES
    nc = build_nc(S=S, n_cores=n_cores)
    in_maps = _prep_host_inputs(queries, keys, values, Wq, bq, Wk, bk, Wv, bv,
                                Wo, bo, S, n_cores)
    res = run_bass_kernel_spmd(nc, in_maps, core_ids=list(range(n_cores)))
    _LAST_RESULT = res
    pieces = [np.asarray(res.results[c]["out"], np.float32)
              for c in range(n_cores)]
    return assemble_out(pieces, S, n_cores).reshape(B, S, D)


def assemble_out(pieces, S, n_cores):
    """Core c's out rows [b*PR + r] hold global tokens b*S + c*PR + r."""
    PR = S // n_cores
    full = np.empty((B * S, D), np.float32)
    for c in range(n_cores):
        oc = pieces[c]
        for b in range(B):
            full[b * S + c * PR:b * S + (c + 1) * PR] = oc[b * PR:(b + 1) * PR]
    return full
